# revision 19
# baseline (speedup 1.0000x reference)
"""Trainium2 Bass kernel for dual-branch local+dilated windowed attention.

Problem: B=1, L=4096, D=512, H=8 heads (dh=64), window=+-256, dilation=4.
reference returns (out_local, out_dilated), each [1, L, D] fp32.

Sharding: sequence (L) sharded across 8 cores; each core owns 512 query rows
and loads a 1024-row key slice (256-row halo each side, zero-padded at the
sequence edges).  All weights are replicated, pre-transposed, and cast to
bf16 host-side with the rmsnorm gains (and the 1/sqrt(dh) score scale)
folded in.  Per-core key-validity masks fold sequence-edge padding and the
key_padding_mask into the softmax denominator via a masked ones-column
appended to V.

On-chip pipeline per core (single NEFF, SPMD over 8 cores):
  1. rmsnorm(x) -> xhat (bf16), PE-transpose to xhat^T [D_part, t]
  2. Q/K/V projections per branch (PE, bf16, fp32 PSUM accum)
  3. scores^T per key chunk = K^T_chunk x Q (two heads of a pair issued
     back-to-back into distinct PE row groups so they run concurrently)
     -> exp (ACT) -> edge-triangle {0,1} mask-mults (DVE, merged 3D APs)
  4. AV with V|colmask STATIONARY and exp^T moving, all key chunks
     accumulated into one [65, 512] PSUM bank per head; row 64 is the
     softmax denominator
  5. normalize: reciprocal of denom row (DVE) -> PE rank-1 broadcast to
     64 partitions -> one DVE multiply writes O^T straight into Wo's
     input layout (no per-tile transposes)
  6. Wo projection, DMA out (dilated rows via a strided view)
"""

import numpy as np
import ml_dtypes

L, D, H, DH = 4096, 512, 8, 64
WIN, DIL = 256, 4
EPS = 1e-6
NCORES = 8
QL = L // NCORES          # 512 queries per core
KL = QL + 2 * WIN         # 1024 keys per core (halo)
P = 128
NKC = KL // P             # 8 key chunks
NQT = QL // P             # 4 query tiles
BF16 = ml_dtypes.bfloat16

_STATE = {}


def _build_nc():
    import concourse.bacc as bacc
    import concourse.tile as tile
    import concourse.mybir as mybir
    from concourse.bass_types import AP
    from concourse.masks import make_identity

    f32 = mybir.dt.float32
    bf16 = mybir.dt.bfloat16
    Exp = mybir.ActivationFunctionType.Exp
    Square = mybir.ActivationFunctionType.Square
    Sqrt = mybir.ActivationFunctionType.Sqrt

    nc = bacc.Bacc()

    xn = nc.dram_tensor("xn", [KL, D], bf16, kind="ExternalInput")
    wT = {}
    for br in ("l", "d"):
        for w in ("wq", "wk", "wv", "wo"):
            wT[w, br] = nc.dram_tensor(f"{w}T_{br}", [D, D], bf16,
                                       kind="ExternalInput")
    tri_lo_d = nc.dram_tensor("tri_lo", [P, P], bf16, kind="ExternalInput")
    tri_hi_d = nc.dram_tensor("tri_hi", [P, P], bf16, kind="ExternalInput")
    colmask_d_ = {
        "l": nc.dram_tensor("colmask_l", [P, NKC], f32, kind="ExternalInput"),
        "d": nc.dram_tensor("colmask_d", [P, NKC], f32, kind="ExternalInput"),
    }
    out_dram = {
        "l": nc.dram_tensor("out_l", [QL, D], f32, kind="ExternalOutput"),
        "d": nc.dram_tensor("out_d", [QL, D], f32, kind="ExternalOutput"),
    }

    def sview(t_ap, off, dims):
        """Raw strided view of an SBUF tile: keep partition dim, custom
        free dims [(stride_elems, n), ...]."""
        return AP(t_ap.tensor, t_ap.offset + off,
                  [t_ap.ap[0]] + [[s, n] for s, n in dims])

    with tile.TileContext(nc) as tc:
        with (
            tc.tile_pool(name="singles", bufs=1) as singles,
            tc.tile_pool(name="xpool", bufs=3) as xpool,
            tc.tile_pool(name="small", bufs=4) as small,
            tc.tile_pool(name="expl", bufs=3) as expool_l,
            tc.tile_pool(name="expd", bufs=2) as expool_d,
            tc.tile_pool(name="outpool", bufs=2) as outpool,
            tc.tile_pool(name="prc", bufs=1, space="PSUM") as psum_rc,
            tc.tile_pool(name="pproj", bufs=2, space="PSUM") as psum_proj,
            tc.tile_pool(name="pst", bufs=3, space="PSUM") as psum_st,
            tc.tile_pool(name="patt", bufs=2, space="PSUM") as psum_att,
        ):
            identity = singles.tile([P, P], bf16)
            make_identity(nc, identity)
            xhatT = singles.tile([P, 4, KL], bf16, name="xhatT")
            eps_t = singles.tile([P, 1], f32, name="eps")
            nc.vector.memset(eps_t, EPS)
            ones_col = singles.tile([1, DH], bf16, name="ones_col")
            nc.vector.memset(ones_col, 1.0)

            # weights/masks on the gpsimd DGE queue, issued from t=0 in
            # parallel with the x tiles on the sync queue.
            tri_lo = singles.tile([P, P], bf16)
            nc.gpsimd.dma_start(tri_lo, tri_lo_d[:, :])
            tri_hi = singles.tile([P, P], bf16)
            nc.gpsimd.dma_start(tri_hi, tri_hi_d[:, :])
            colmask = {}
            for br in ("l", "d"):
                colmask[br] = singles.tile([P, NKC], f32, name=f"cm_{br}")
                nc.gpsimd.dma_start(colmask[br], colmask_d_[br][:, :])
            w_sb = {}
            for (w, br), dt_ in wT.items():
                w_sb[w, br] = singles.tile([P, 4, D], bf16, name=f"{w}_{br}")
                nc.gpsimd.dma_start(
                    w_sb[w, br],
                    dt_[:, :].rearrange("(ic p) o -> p ic o", p=P),
                )

            # ---- rmsnorm + transpose ----
            # all x-tile DMAs issued up front (8-deep ring) so the last
            # tiles aren't gated behind rms compute on the scalar queue
            xts = {}
            for i, tt in enumerate((2, 3, 4, 5, 0, 1, 6, 7)):
                xts[tt] = xpool.tile([P, D], bf16, tag="xt", bufs=8,
                                     name="xt")
                dma_eng = nc.sync if i % 2 == 0 else nc.scalar
                dma_eng.dma_start(xts[tt], xn[tt * P:(tt + 1) * P, :])
            for tt in (2, 3, 4, 5, 0, 1, 6, 7):
                xt = xts[tt]
                sqd = xpool.tile([P, D], bf16, tag="sqd")
                ssum = small.tile([P, 1], f32, tag="ssum")
                nc.scalar.activation(sqd, xt, Square, accum_out=ssum)
                rstd = small.tile([P, 1], f32, tag="rstd")
                nc.scalar.activation(rstd, ssum, Sqrt, bias=eps_t, scale=1.0 / D)
                nc.vector.reciprocal(rstd, rstd)
                xh = xpool.tile([P, D], bf16, tag="xh")
                nc.vector.tensor_scalar_mul(xh, xt, rstd)
                for ic in range(4):
                    tp = psum_st.tile([P, P], bf16, tag="st", name="tp")
                    nc.tensor.transpose(tp, xh[:, ic * P:(ic + 1) * P], identity)
                    if ic % 2 == 0:
                        nc.vector.tensor_copy(
                            xhatT[:, ic, tt * P:(tt + 1) * P], tp)
                    else:
                        nc.scalar.copy(xhatT[:, ic, tt * P:(tt + 1) * P], tp)

            QT, KT, V, OT = {}, {}, {}, {}
            for br in ("l", "d"):
                QT[br] = singles.tile([P, 4, QL], bf16, name=f"QT_{br}")
                KT[br] = singles.tile([P, 4, KL], bf16, name=f"KT_{br}")
                V[br] = singles.tile([P, NKC, H, DH + 1], bf16, name=f"V_{br}")
                OT[br] = singles.tile([P, 4, QL], bf16, name=f"OT_{br}")

            def key_cols_ap(ic, kc, br):
                # lhsT [128, 128] of xhat^T columns for key chunk kc
                if br == "l":
                    return xhatT[:, ic, kc * P:(kc + 1) * P]
                rho, s = kc // 2, kc % 2
                return xhatT[:, ic, :].rearrange(
                    "p (b four) -> p four b", four=DIL)[:, rho, s * P:(s + 1) * P]

            # ---- projections ----
            for br in ("l", "d"):
                for pair in range(4):
                    ps = psum_proj.tile([P, D], f32, tag="pp")
                    for ic in range(4):
                        nc.tensor.matmul(
                            ps, w_sb["wq", br][:, ic, pair * P:(pair + 1) * P],
                            xhatT[:, ic, WIN:WIN + QL],
                            start=(ic == 0), stop=(ic == 3))
                    if pair % 2 == 0:
                        nc.vector.tensor_copy(QT[br][:, pair, :], ps)
                    else:
                        nc.scalar.copy(QT[br][:, pair, :], ps)
                for pair in range(4):
                    for half in range(2):
                        ps = psum_proj.tile([P, D], f32, tag="pp")
                        for ic in range(4):
                            nc.tensor.matmul(
                                ps, w_sb["wk", br][:, ic, pair * P:(pair + 1) * P],
                                xhatT[:, ic, half * D:(half + 1) * D],
                                start=(ic == 0), stop=(ic == 3))
                        if half == 0:
                            nc.vector.tensor_copy(
                                KT[br][:, pair, half * D:(half + 1) * D], ps)
                        else:
                            nc.scalar.copy(
                                KT[br][:, pair, half * D:(half + 1) * D], ps)
                # ones-columns of V: colmask broadcast over kc's heads
                cm = colmask[br]
                cm_b = AP(cm[:, :].tensor, cm[:, :].offset,
                          [cm[:, :].ap[0], [1, NKC], [0, H], [0, 1]])
                nc.gpsimd.tensor_copy(V[br][:, :, :, DH:DH + 1], cm_b)
                for kc in range(NKC):
                    ps = psum_proj.tile([P, D], f32, tag="pp")
                    for ic in range(4):
                        nc.tensor.matmul(
                            ps, key_cols_ap(ic, kc, br), w_sb["wv", br][:, ic, :],
                            start=(ic == 0), stop=(ic == 3))
                    nc.vector.tensor_scalar_mul(
                        V[br][:, kc, :, 0:DH],
                        ps.rearrange("p (h dv) -> p h dv", h=H),
                        colmask[br][:, kc:kc + 1])

            # ---- attention ----
            def qrange(kc):
                return max(0, P * (kc - 4)), min(QL, P * kc + P)

            def divide_and_store(br, pair, hh, opT):
                """denominator recip -> PE rank-1 broadcast -> one DVE
                multiply writes OT[r0:r0+64, pair, :]."""
                r0 = 64 * hh
                rcp = small.tile([1, QL], bf16, tag="rcp")
                with nc.allow_low_precision("softmax denom recip in bf16"):
                    nc.vector.reciprocal(rcp, opT[DH:DH + 1, :])
                rcpb = psum_rc.tile([DH, QL], f32, tag="rcpb")
                nc.tensor.matmul(rcpb, ones_col[0:1, :], rcp[0:1, :])
                rcps = small.tile([DH, QL], bf16, tag="rcps")
                if hh == 0:
                    nc.vector.tensor_copy(rcps, rcpb)
                else:
                    nc.scalar.copy(rcps, rcpb)
                nc.vector.tensor_mul(
                    OT[br][r0:r0 + 64, pair, :], opT[0:DH, :], rcps)

            for br, pair in (("l", 0), ("l", 1), ("d", 0), ("l", 2),
                             ("d", 1), ("l", 3), ("d", 2), ("d", 3)):
                    if br == "l":
                        ex = [expool_l.tile([P, NKC, QL], bf16, tag=f"exl{hh}",
                                            name=f"exl{hh}")
                              for hh in range(2)]
                        # scores: hh pairs back-to-back -> concurrent PE
                        # row groups (tile_position (0,0) / (64,0))
                        for kc in range(NKC):
                            qlo, qhi = qrange(kc)
                            n = qhi - qlo
                            sts = []
                            for hh in range(2):
                                r0 = 64 * hh
                                st = psum_st.tile([P, QL], f32, tag="st")
                                nc.tensor.matmul(
                                    st[:, :n],
                                    KT[br][r0:r0 + 64, pair, kc * P:(kc + 1) * P],
                                    QT[br][r0:r0 + 64, pair, qlo:qhi])
                                sts.append(st)
                            for hh in range(2):
                                nc.scalar.activation(
                                    ex[hh][:, kc, qlo:qhi], sts[hh][:, :n], Exp)
                        # edge triangle masks, 4 chunks per op via strided APs
                        for hh in range(2):
                            e = ex[hh][:, :, :]
                            lo = sview(e, 0, [(QL + P, 4), (1, P)])
                            hi = sview(e, 4 * QL, [(QL + P, 4), (1, P)])
                            tlo = sview(tri_lo[:, :], 0, [(0, 4), (1, P)])
                            thi = sview(tri_hi[:, :], 0, [(0, 4), (1, P)])
                            eng = nc.vector if hh == 0 else nc.gpsimd
                            eng.tensor_mul(lo, lo, tlo)
                            eng.tensor_mul(hi, hi, thi)
                        # AV: V|colmask stationary, ex moving; all chunks
                        # into one [65, 512] PSUM bank; row 64 = denom
                        for hh in range(2):
                            h = 2 * pair + hh
                            opT = psum_att.tile([DH + 1, QL], f32, tag="opT")
                            for i, kc in enumerate((4, 0, 1, 2, 3, 5, 6, 7)):
                                qlo, qhi = qrange(kc)
                                nc.tensor.matmul(
                                    opT[:, qlo:qhi], V[br][:, kc, h, :],
                                    ex[hh][:, kc, qlo:qhi],
                                    start=(i == 0), stop=(i == 7),
                                    skip_group_check=True)
                            divide_and_store(br, pair, hh, opT)
                    else:
                        ex = [expool_d.tile([P, NKC, P], bf16, tag=f"exd{hh}",
                                            name=f"exd{hh}")
                              for hh in range(2)]
                        for half in range(2):
                            sts = []
                            for hh in range(2):
                                r0 = 64 * hh
                                st = psum_st.tile([P, QL], f32, tag="st", name="st")
                                sts.append(st)
                            for j in range(4):
                                idx = half * 4 + j
                                rho, s = idx // 2, idx % 2
                                for hh in range(2):
                                    r0 = 64 * hh
                                    ktv = KT[br][r0:r0 + 64, pair, :].rearrange(
                                        "p (b four) -> p four b", four=DIL
                                    )[:, rho, s * P:(s + 1) * P]
                                    qtv = QT[br][r0:r0 + 64, pair, :].rearrange(
                                        "p (a four) -> p four a", four=DIL)[:, rho, :]
                                    nc.tensor.matmul(
                                        sts[hh][:, j * P:(j + 1) * P], ktv, qtv)
                            for hh in range(2):
                                nc.scalar.activation(
                                    ex[hh][:, half * 4:(half + 1) * 4, :],
                                    sts[hh], Exp)
                        for hh in range(2):
                            e = ex[hh][:, :, :]
                            lo = sview(e, 0, [(2 * P, 4), (1, P)])
                            hi = sview(e, P, [(2 * P, 4), (1, P)])
                            tlo = sview(tri_lo[:, :], 0, [(0, 4), (1, P)])
                            thi = sview(tri_hi[:, :], 0, [(0, 4), (1, P)])
                            eng = nc.vector if hh == 0 else nc.gpsimd
                            eng.tensor_mul(lo, lo, tlo)
                            eng.tensor_mul(hi, hi, thi)
                        for hh in range(2):
                            h = 2 * pair + hh
                            opT = psum_att.tile([DH + 1, QL], f32, tag="opT")
                            for rho in range(DIL):
                                for s in range(2):
                                    nc.tensor.matmul(
                                        opT[:, rho * P:(rho + 1) * P],
                                        V[br][:, rho * 2 + s, h, :],
                                        ex[hh][:, rho * 2 + s, :],
                                        start=(rho == 0 and s == 0),
                                        stop=(rho == 3 and s == 1),
                                        skip_group_check=True)
                            divide_and_store(br, pair, hh, opT)

                # ---- Wo ----
                for t in range(NQT):
                    ps = psum_proj.tile([P, D], f32, tag="pp")
                    for pair in range(4):
                        nc.tensor.matmul(
                            ps, OT[br][:, pair, t * P:(t + 1) * P],
                            w_sb["wo", br][:, pair, :],
                            start=(pair == 0), stop=(pair == 3))
                    ob = outpool.tile([P, D], f32, tag="ob")
                    nc.vector.tensor_copy(ob, ps)
                    if br == "l":
                        nc.sync.dma_start(out_dram[br][t * P:(t + 1) * P, :], ob)
                    else:
                        dst = out_dram[br][:, :].rearrange(
                            "(a four) o -> four a o", four=DIL)[t]
                        nc.sync.dma_start(dst, ob)

    nc.finalize()
    return nc


def _prep_host(x, key_padding_mask, weights):
    """Build the per-core input maps (weights shared across cores)."""
    x = np.asarray(x, dtype=np.float32).reshape(L, D)
    kpm = np.asarray(key_padding_mask).reshape(L).astype(bool)

    shared = {}
    for name, arr in weights.items():
        shared[name] = np.ascontiguousarray(arr.T).astype(BF16)

    idx = np.arange(P)
    tri_lo = (idx[:, None] >= idx[None, :]).astype(BF16)
    tri_hi = (idx[:, None] <= idx[None, :]).astype(BF16)
    shared["tri_lo"], shared["tri_hi"] = tri_lo, tri_hi

    valid_full = np.zeros(L + 2 * WIN, dtype=np.float32)
    valid_full[WIN:WIN + L] = (~kpm).astype(np.float32)

    in_maps = []
    for c in range(NCORES):
        lo = c * QL - WIN
        xnc = np.zeros((KL, D), dtype=np.float32)
        a, b = max(lo, 0), min(lo + KL, L)
        xnc[a - lo:b - lo] = x[a:b]
        v = valid_full[lo + WIN:lo + WIN + KL]  # validity of keys lo..lo+KL
        cm_l = v.reshape(NKC, P).T.astype(np.float32)
        # dilated chunk idx = rho*2+s holds keys lk = 4*(128*s + p) + rho
        cm_d = np.empty((P, NKC), dtype=np.float32)
        for rho in range(DIL):
            for s in range(2):
                lk = DIL * (P * s + idx) + rho
                cm_d[:, rho * 2 + s] = v[lk]
        m = dict(shared)
        m["xn"] = xnc.astype(BF16)
        m["colmask_l"] = np.ascontiguousarray(cm_l)
        m["colmask_d"] = np.ascontiguousarray(cm_d)
        in_maps.append(m)
    return in_maps


def kernel(x, key_padding_mask, wq_l, wk_l, wv_l, wo_l,
           wq_d, wk_d, wv_d, wo_d, g_q, g_kv, **run_kwargs):
    from concourse.bass_utils import run_bass_kernel_spmd

    g_q = np.asarray(g_q, dtype=np.float32)
    g_kv = np.asarray(g_kv, dtype=np.float32)
    scale = 1.0 / np.sqrt(DH)
    weights = {
        "wqT_l": np.asarray(wq_l, np.float32) * (g_q * scale)[None, :],
        "wkT_l": np.asarray(wk_l, np.float32) * g_kv[None, :],
        "wvT_l": np.asarray(wv_l, np.float32) * g_kv[None, :],
        "woT_l": np.asarray(wo_l, np.float32),
        "wqT_d": np.asarray(wq_d, np.float32) * (g_q * scale)[None, :],
        "wkT_d": np.asarray(wk_d, np.float32) * g_kv[None, :],
        "wvT_d": np.asarray(wv_d, np.float32) * g_kv[None, :],
        "woT_d": np.asarray(wo_d, np.float32),
    }
    in_maps = _prep_host(x, key_padding_mask, weights)

    if "nc" not in _STATE:
        _STATE["nc"] = _build_nc()
    res = run_bass_kernel_spmd(_STATE["nc"], in_maps,
                               core_ids=list(range(NCORES)), **run_kwargs)
    _STATE["last_result"] = res

    out_l = np.concatenate([res.results[c]["out_l"] for c in range(NCORES)],
                           axis=0).reshape(1, L, D)
    out_d = np.concatenate([res.results[c]["out_d"] for c in range(NCORES)],
                           axis=0).reshape(1, L, D)
    return (out_l, out_d)
